# revision 12
# baseline (speedup 1.0000x reference)
"""Multi-head attention (B=2, S=4096, D=768, H=12) on 8 TRN2 NeuronCores.

Sharding: 24 (batch, head) pairs -> 3 heads per core. Cores 0-3 take batch 0,
cores 4-7 take batch 1. Each core computes q/k/v projections for its 3 heads,
flash-style attention (scores kept transposed [kv, q] so exp can run straight
out of PSUM), and a partial output projection over its 192 contraction rows.
The host sums the 4 partial outputs per batch and adds the output bias.

v2 structure:
- Inputs land as one 768 KiB DMA per 512-seq block (3 KiB/partition lines),
  alternating between the two HWDGE issue engines (sync / scalar).
- All projections (k, v, q) run up front; attention then owns all 8 PSUM
  banks: 2x [128,1536] score tiles + ctx accumulator + out-proj staging.
- Softmax exp is split across two engines: scalar ACT computes true exp
  (fp8 out), and the vector engine computes a Schraudolph-style exp for
  ~1/3 of the kv chunks: u8 = round(s*A + B) bit-cast as fp8e4m3, a
  piecewise-linear 2^x approximation (~3% max err, zero-mean).
- P@V runs in fp8 with DoubleRow perf mode: each matmul contracts TWO kv
  chunks (virtual 256-deep array), halving P@V tensor-engine time. The
  denominator falls out via a ones-column appended to V.
"""

import sys

sys.path.insert(0, "/opt/trn_rl_repo")

import numpy as np  # noqa: E402

from concourse import bacc, bass, mybir, tile  # noqa: E402
from concourse.bass_utils import run_bass_kernel_spmd  # noqa: E402

S = 4096
DM = 768
DK = 64
HPC = 3  # heads per core
NC_CORES = 8
KC = DM // 128  # 6 contraction chunks for projections
NSB = S // 512  # 8 seq blocks (projection N / attention q chunks)
NKV = S // 128  # 32 kv chunks
SCALE = 1.0 / np.sqrt(DK)
# Schraudolph fp8e4m3 exp: u8 = round(s * A + B), bits viewed as fp8.
A_SCH = float(8.0 * np.log2(np.e) * SCALE)
B_SCH = float(56.0 - 0.3443)
DVE_PAIRS = (1, 4, 6, 9, 11, 14)  # exp pairs computed on the vector engine

F16 = mybir.dt.float16
F32 = mybir.dt.float32
F8 = mybir.dt.float8e4
U8 = mybir.dt.uint8
DR = mybir.MatmulPerfMode.DoubleRow


def _emit(tc):
    nc = tc.nc
    qTx = nc.dram_tensor("qTx", [NSB, 128, KC, 512], F16, kind="ExternalInput").ap()
    kTx = nc.dram_tensor("kTx", [NSB, 128, KC, 512], F16, kind="ExternalInput").ap()
    vTx = nc.dram_tensor("vTx", [NSB, 128, KC, 512], F16, kind="ExternalInput").ap()
    wqT = nc.dram_tensor("wqT", [DM, HPC * DK], F16, kind="ExternalInput").ap()
    wkT = nc.dram_tensor("wkT", [DM, HPC * DK], F16, kind="ExternalInput").ap()
    wvT = nc.dram_tensor("wvT", [DM, HPC * DK], F16, kind="ExternalInput").ap()
    woT = nc.dram_tensor("woT", [HPC * DK, DM], F16, kind="ExternalInput").ap()
    bq = nc.dram_tensor("bq", [HPC * DK, 1], F32, kind="ExternalInput").ap()
    bk = nc.dram_tensor("bk", [HPC * DK, 1], F32, kind="ExternalInput").ap()
    bv = nc.dram_tensor("bv", [HPC * DK, 1], F32, kind="ExternalInput").ap()
    out_p = nc.dram_tensor("out_p", [S, DM], F16, kind="ExternalOutput").ap()
    den_d = nc.dram_tensor("den_d", [NSB * HPC, 512], F32, kind="Internal").ap()

    with (
        tc.tile_pool(name="const", bufs=1) as const,
        tc.tile_pool(name="heads", bufs=1) as heads,
        tc.tile_pool(name="xts", bufs=10) as xts,
        tc.tile_pool(name="work", bufs=3) as work,
        tc.tile_pool(name="norm", bufs=4) as norm,
    ):
        # ---- constants -------------------------------------------------
        w_q = const.tile([128, KC, HPC * DK], F16, tag="w_q")
        w_k = const.tile([128, KC, HPC * DK], F16, tag="w_k")
        w_v = const.tile([128, KC, HPC * DK], F16, tag="w_v")
        nc.sync.dma_start(w_k[:], wkT.rearrange("(c p) m -> p c m", p=128))
        wo01 = const.tile([128, DM], F16, tag="wo01")
        wo2 = const.tile([DK, DM], F16, tag="wo2")
        bq01 = const.tile([128, 1], F32, tag="bq01")
        bq2 = const.tile([DK, 1], F32, tag="bq2")
        bk01 = const.tile([128, 1], F32, tag="bk01")
        bk2 = const.tile([DK, 1], F32, tag="bk2")
        nc.sync.dma_start(bk01[:], bk[0:128, :])
        nc.sync.dma_start(bk2[:], bk[128:192, :])
        # v-bias broadcast to all 128 partitions: bvb[p, j] = bv[j]
        bvb = const.tile([128, HPC * DK], F32, tag="bvb")
        bv_bcast = bass.AP(
            tensor=bv.tensor, offset=bv.offset, ap=[[0, 128]] + list(bv.ap)
        )

        # preload the exp activation table during the projection phase
        warm = const.tile([1, 1], F32, tag="warm")
        nc.vector.memset(warm[:], 0.0)
        nc.scalar.activation(warm[:], warm[:], mybir.ActivationFunctionType.Exp)

        # ---- per-head persistent tensors ------------------------------
        # qT2/kT2: [128, S] fp16, rows 0:64 and 64:128 both hold head's
        # qT/kT (duplicated so row-tiled matmul pairs can stream from
        # either partition half).
        qT2 = [heads.tile([128, S], F16, tag=f"qT2_{h}", name=f"qT2_{h}") for h in range(HPC)]
        kT2 = [heads.tile([128, S], F16, tag=f"kT2_{h}", name=f"kT2_{h}") for h in range(HPC)]
        # v_dr: [128, NKV, 80] fp8; [p, g, 0:64] = v of kv chunk g (seq pos p
        # on partitions), col 64 = 1.0 (denominator column), 65:80 pad.
        v_dr = [heads.tile([128, NKV, 80], F8, tag=f"v_dr_{h}", name=f"v_dr_{h}") for h in range(HPC)]
        for h in range(HPC):
            nc.vector.memset(v_dr[h][:], 1.0)
        # normalized context, transposed: ctx01 rows 0:64 = head 0, rows
        # 64:128 = head 1; ctx2 = head 2. Together the lhsT of the output
        # projection.
        ctx01 = heads.tile([128, S], F16, tag="ctx01")
        ctx2 = heads.tile([64, S], F16, tag="ctx2")

        def dma_in(tile_ap, src, i):
            eng = nc.sync if i % 2 == 0 else nc.scalar
            eng.dma_start(tile_ap, src)

        # ---- projections: k --------------------------------------------
        with tc.tile_pool(name="pp", bufs=2, space=bass.MemorySpace.PSUM) as pp:
            kxs = []
            for sb in range(NSB):
                kx = xts.tile([128, KC, 512], F16, tag="xx", name=f"kx_{sb}")
                dma_in(kx[:], kTx[sb], sb)
                kxs.append(kx)
            for sb in range(NSB):
                sq = bass.ts(sb, 512)
                k01 = pp.tile([128, 512], F32, tag="k01")
                k2 = pp.tile([DK, 512], F32, tag="k2")
                for kc in range(KC):
                    st = dict(start=(kc == 0), stop=(kc == KC - 1))
                    xsl = kxs[sb][:, kc, :]
                    nc.tensor.matmul(k01[:], w_k[:, kc, 0:128], xsl, **st)
                    nc.tensor.matmul(k2[:], w_k[:, kc, 128:192], xsl, **st)
                nc.vector.tensor_scalar_add(kT2[0][0:64, sq], k01[0:64, :], bk01[0:64, :])
                nc.vector.tensor_scalar_add(kT2[1][0:64, sq], k01[64:128, :], bk01[64:128, :])
                nc.vector.tensor_scalar_add(kT2[2][0:64, sq], k2[:], bk2[:])
                for h in range(HPC):
                    nc.sync.dma_start(kT2[h][64:128, sq], kT2[h][0:64, sq])

        # v-proj inputs early so the DMA stream stays busy
        nc.sync.dma_start(w_v[:], wvT.rearrange("(c p) m -> p c m", p=128))
        nc.sync.dma_start(bvb[:], bv_bcast)

        # ---- projections: v -------------------------------------------
        # v rows (seq) on partitions: out tile [128 seq, 192] per kv chunk.
        with tc.tile_pool(name="vp", bufs=4, space=bass.MemorySpace.PSUM) as vp:
            vxs = []
            for sb in range(NSB):
                vx = xts.tile([128, KC, 512], F16, tag="xx", name=f"vx_{sb}")
                dma_in(vx[:], vTx[sb], sb)
                vxs.append(vx)
            for sb in range(NSB):
                for ss in range(4):  # kv chunk index = 4*sb + ss
                    vps = vp.tile([128, HPC * DK], F32, tag="vps")
                    for kc in range(KC):
                        nc.tensor.matmul(
                            vps[:],
                            vxs[sb][:, kc, bass.ds(ss * 128, 128)],
                            w_v[:, kc, :],
                            start=(kc == 0),
                            stop=(kc == KC - 1),
                        )
                    g = 4 * sb + ss
                    for h in range(HPC):
                        nc.vector.tensor_add(
                            v_dr[h][:, g, 0:64],
                            vps[:, bass.ts(h, 64)],
                            bvb[:, bass.ts(h, 64)],
                        )

        # remaining constant loads
        nc.sync.dma_start(w_q[:], wqT.rearrange("(c p) m -> p c m", p=128))
        nc.sync.dma_start(bq01[:], bq[0:128, :])
        nc.sync.dma_start(bq2[:], bq[128:192, :])
        nc.sync.dma_start(wo01[:], woT[0:128, :])
        nc.sync.dma_start(wo2[:], woT[128:192, :])

        # ---- attention + output projection ----------------------------
        # q chunks outer, heads inner. Scores land in [128, 1024] PSUM tiles
        # (one kv-chunk pair per exp, 3-deep ring shared with the q-proj
        # accumulator); P is written as fp8 into flat [128, 1024] staging
        # tiles, consumed by one DoubleRow P@V matmul per pair, issued two
        # slots late so its exp is long finished. The next q chunk's
        # projection rides inside head 2's loop; the previous q chunk's
        # output projection inside head 0's. Each head's normalize
        # (reciprocal + scale) is deferred into the NEXT head's loop so the
        # denominator's DRAM-broadcast round trip never blocks the DVE queue.
        with (
            tc.tile_pool(name="sp", bufs=3, space=bass.MemorySpace.PSUM) as sp,
            tc.tile_pool(name="bigp", bufs=2, space=bass.MemorySpace.PSUM) as bigp,
        ):
            def op_chain(qc, i):
                # one eighth of q-chunk qc's output projection
                qs, half = i // 2, i % 2
                n0, nw = (0, 512) if half == 0 else (512, 256)
                qsl = bass.ds(qc * 512 + qs * 128, 128)
                op = bigp.tile([128, 512], F32, tag="big", name=f"op_{qc}_{i}")
                nc.tensor.matmul(
                    op[:, 0:nw], ctx01[:, qsl], wo01[:, n0 : n0 + nw],
                    start=True, stop=False,
                )
                nc.tensor.matmul(
                    op[:, 0:nw], ctx2[:, qsl], wo2[:, n0 : n0 + nw],
                    start=False, stop=True,
                )
                ob = work.tile([128, 512], F16, tag="ob", name=f"ob_{qc}_{i}")
                nc.vector.tensor_copy(ob[:, 0:nw], op[:, 0:nw])
                nc.sync.dma_start(out_p[qsl, n0 : n0 + nw], ob[:, 0:nw])

            qp_state = {}

            def qproj_step(qc, kc):
                if kc == 0:
                    qp_state[qc] = sp.tile([128, 1024], F32, tag="sT",
                                           name=f"qp_{qc}")
                    qx = xts.tile([128, KC, 512], F16, tag="xx",
                                  name=f"qx_{qc}")
                    nc.sync.dma_start(qx[:], qTx[qc])
                    qp_state[f"x{qc}"] = qx
                qp = qp_state[qc]
                qx = qp_state[f"x{qc}"]
                st = dict(start=(kc == 0), stop=(kc == KC - 1))
                xsl = qx[:, kc, :]
                nc.tensor.matmul(qp[:, 0:512], w_q[:, kc, 0:128], xsl, **st)
                nc.tensor.matmul(qp[0:64, 512:1024], w_q[:, kc, 128:192], xsl, **st)

            def qproj_drain(qc):
                sq = bass.ts(qc, 512)
                qp = qp_state.pop(qc)
                qp_state.pop(f"x{qc}")
                nc.vector.tensor_scalar_add(qT2[0][0:64, sq], qp[0:64, 0:512], bq01[0:64, :])
                nc.vector.tensor_scalar_add(qT2[1][0:64, sq], qp[64:128, 0:512], bq01[64:128, :])
                nc.vector.tensor_scalar_add(qT2[2][0:64, sq], qp[0:64, 512:1024], bq2[:])
                for h in range(HPC):
                    nc.sync.dma_start(qT2[h][64:128, sq], qT2[h][0:64, sq])

            pending_norm = [None]

            def flush_norm():
                if pending_norm[0] is not None:
                    pending_norm[0]()
                    pending_norm[0] = None

            for kc in range(KC):
                qproj_step(0, kc)
            qproj_drain(0)

            prev = [None]  # (ctx_mm, finish) of the previous head

            def finish_head(qc, h, ctx):
                # denominator row -> SBUF (on ACT) -> DRAM -> stride-0
                # broadcast back to 64 partitions; reciprocal + scale are
                # deferred further (flush_norm) so the round trip never
                # blocks the DVE queue.
                sq = bass.ts(qc, 512)
                den_row = norm.tile([1, 512], F32, tag="den_row")
                nc.scalar.copy(den_row[:], ctx[64:65, :])
                di = qc * HPC + h
                nc.sync.dma_start(den_d[di : di + 1, :], den_row[:])
                den = norm.tile([64, 512], F32, tag="den")
                dsrc = den_d[di : di + 1, :]
                den_bcast = bass.AP(
                    tensor=dsrc.tensor,
                    offset=dsrc.offset,
                    ap=[[0, 64]] + list(dsrc.ap[1:]),
                )
                nc.sync.dma_start(den[:], den_bcast)

                def normalize():
                    rec = norm.tile([64, 512], F32, tag="rec")
                    nc.vector.reciprocal_approx_fast(out=rec[:], in_=den[:])
                    if h == 0:
                        nc.vector.tensor_mul(ctx01[0:64, sq], ctx[0:64, :], rec[:])
                    elif h == 1:
                        nc.vector.tensor_mul(ctx01[64:128, sq], ctx[0:64, :], rec[:])
                    else:
                        nc.vector.tensor_mul(ctx2[:, sq], ctx[0:64, :], rec[:])

                pending_norm[0] = normalize

            for qc in range(NSB):
                sq = bass.ts(qc, 512)
                for h in range(HPC):
                    ctx_t = bigp.tile([128, 512], F32, tag="big")
                    ctx = ctx_t[0:65, :]
                    pts = {}

                    def ctx_mm(g, ctx=ctx, h=h, pts=pts):
                        nc.tensor.matmul(
                            ctx,
                            v_dr[h][:, 2 * g : 2 * g + 2, 0:65],
                            pts.pop(g)[:].rearrange("p (a b) -> p a b", a=2),
                            start=(g == 0), stop=(g == 15),
                            perf_mode=DR,
                        )

                    for g in range(16):  # kv-chunk pairs
                        # previous head's last two P@V matmuls ride in this
                        # head's first two slots (no PE drain at the boundary)
                        if g <= 1 and prev[0] is not None:
                            prev[0][0](14 + g)
                            if g == 1:
                                prev[0][1]()
                                prev[0] = None
                        if g >= 2:
                            ctx_mm(g - 2)
                        sT = sp.tile([128, 1024], F32, tag="sT")
                        for j in (0, 1):
                            c = 2 * g + j
                            lo = 64 * j
                            nc.tensor.matmul(
                                sT[:, bass.ts(j, 512)],
                                kT2[h][lo : lo + 64, bass.ts(c, 128)],
                                qT2[h][lo : lo + 64, sq],
                            )
                        pt = work.tile([128, 1024], F8, tag="pt", bufs=6,
                                       name=f"pt_{qc}_{h}_{g}")
                        pts[g] = pt
                        if g in DVE_PAIRS:
                            nc.vector.tensor_scalar(
                                pt.bitcast(U8)[:], sT[:], A_SCH, B_SCH,
                                mybir.AluOpType.mult, mybir.AluOpType.add,
                            )
                        else:
                            nc.scalar.activation(
                                pt[:], sT[:], mybir.ActivationFunctionType.Exp,
                                scale=SCALE,
                            )
                        if g == 6:
                            flush_norm()
                        # previous q-chunk's output projection, spread across
                        # heads 0 and 1
                        if h <= 1 and qc > 0 and g in (7, 9, 11, 13):
                            op_chain(qc - 1, 4 * h + (g - 7) // 2)
                        # next q-chunk's projection inside head 2's loop
                        if h == 2 and qc + 1 < NSB:
                            if 1 <= g <= 6:
                                qproj_step(qc + 1, g - 1)
                            elif g == 7:
                                qproj_drain(qc + 1)
                    prev[0] = (ctx_mm, lambda qc=qc, h=h, ctx=ctx: finish_head(qc, h, ctx))
            # drain the last head
            ctx_mm_f, fin = prev[0]
            ctx_mm_f(14)
            ctx_mm_f(15)
            fin()
            flush_norm()
            # last q-chunk's output projection
            for i in range(8):
                op_chain(NSB - 1, i)


_NC_CACHE = {}


def _build():
    if "nc" not in _NC_CACHE:
        nc = bacc.Bacc(
            "TRN2", target_bir_lowering=False, debug=False, num_devices=NC_CORES
        )
        with tile.TileContext(nc) as tc:
            _emit(tc)
        nc.compile()
        _NC_CACHE["nc"] = nc
    return _NC_CACHE["nc"]


def _tile_xT(x):
    # x: [S, DM] fp32 -> x.T tiled as [NSB, 128, KC*512] fp16: one contiguous
    # 768 KiB block per 512-seq chunk (3 KiB per partition line).
    xT = np.ascontiguousarray(x.T).astype(np.float16)  # [DM, S]
    t = xT.reshape(KC, 128, NSB, 512).transpose(2, 1, 0, 3)
    return np.ascontiguousarray(t)


def make_in_maps(query, key, value, wq, bq, wk, bk, wv, bv, wo, bo):
    query = np.asarray(query)
    key = np.asarray(key)
    value = np.asarray(value)
    wq, bq, wk, bk, wv, bv, wo, bo = (
        np.asarray(a) for a in (wq, bq, wk, bk, wv, bv, wo, bo)
    )
    in_maps = []
    for c in range(NC_CORES):
        b = c // 4
        hs = (c % 4) * HPC * DK
        he = hs + HPC * DK
        in_maps.append(
            {
                "qTx": _tile_xT(query[b]),
                "kTx": _tile_xT(key[b]),
                "vTx": _tile_xT(value[b]),
                "wqT": np.ascontiguousarray(wq[hs:he, :].T).astype(np.float16),
                "wkT": np.ascontiguousarray(wk[hs:he, :].T).astype(np.float16),
                "wvT": np.ascontiguousarray(wv[hs:he, :].T).astype(np.float16),
                "woT": np.ascontiguousarray(wo[:, hs:he].T).astype(np.float16),
                "bq": bq[hs:he].reshape(-1, 1).astype(np.float32),
                "bk": bk[hs:he].reshape(-1, 1).astype(np.float32),
                "bv": bv[hs:he].reshape(-1, 1).astype(np.float32),
            }
        )
    return in_maps


def combine_outputs(results, bo):
    parts = [results[c]["out_p"].astype(np.float32) for c in range(NC_CORES)]
    out0 = parts[0] + parts[1] + parts[2] + parts[3]
    out1 = parts[4] + parts[5] + parts[6] + parts[7]
    out = np.stack([out0, out1]) + np.asarray(bo)[None, None, :]
    return out.astype(np.float32)


def run_on_hw(in_maps, **kw):
    nc = _build()
    return run_bass_kernel_spmd(nc, in_maps, list(range(NC_CORES)), **kw)


def kernel(query, key, value, wq, bq, wk, bk, wv, bv, wo, bo):
    in_maps = make_in_maps(query, key, value, wq, bq, wk, bk, wv, bv, wo, bo)
    res = run_on_hw(in_maps)
    return combine_outputs(res.results, bo)


# revision 13
# speedup vs baseline: 1.0739x; 1.0739x over previous
"""Multi-head attention (B=2, S=4096, D=768, H=12) on 8 TRN2 NeuronCores.

Sharding: 24 (batch, head) pairs -> 3 heads per core. Cores 0-3 take batch 0,
cores 4-7 take batch 1. Each core computes q/k/v projections for its 3 heads,
flash-style attention (scores kept transposed [kv, q] so exp can run straight
out of PSUM), and a partial output projection over its 192 contraction rows.
The host sums the 4 partial outputs per batch and adds the output bias.

v2 structure:
- Inputs land as one 768 KiB DMA per 512-seq block (3 KiB/partition lines),
  alternating between the two HWDGE issue engines (sync / scalar).
- All projections (k, v, q) run up front; attention then owns all 8 PSUM
  banks: 2x [128,1536] score tiles + ctx accumulator + out-proj staging.
- Softmax exp is split across two engines: scalar ACT computes true exp
  (fp8 out), and the vector engine computes a Schraudolph-style exp for
  ~1/3 of the kv chunks: u8 = round(s*A + B) bit-cast as fp8e4m3, a
  piecewise-linear 2^x approximation (~3% max err, zero-mean).
- P@V runs in fp8 with DoubleRow perf mode: each matmul contracts TWO kv
  chunks (virtual 256-deep array), halving P@V tensor-engine time. The
  denominator falls out via a ones-column appended to V.
"""

import sys

sys.path.insert(0, "/opt/trn_rl_repo")

import numpy as np  # noqa: E402

from concourse import bacc, bass, mybir, tile  # noqa: E402
from concourse.bass_utils import run_bass_kernel_spmd  # noqa: E402

S = 4096
DM = 768
DK = 64
HPC = 3  # heads per core
NC_CORES = 8
KC = DM // 128  # 6 contraction chunks for projections
NSB = S // 512  # 8 seq blocks (projection N / attention q chunks)
NKV = S // 128  # 32 kv chunks
SCALE = 1.0 / np.sqrt(DK)
# Schraudolph fp8e4m3 exp: u8 = round(s * A + B), bits viewed as fp8.
A_SCH = float(8.0 * np.log2(np.e) * SCALE)
B_SCH = float(56.0 - 0.3443)
DVE_PAIRS = (1, 3, 5, 7, 9, 11, 13, 15)  # exp pairs computed on the vector engine

F16 = mybir.dt.float16
F32 = mybir.dt.float32
F8 = mybir.dt.float8e4
U8 = mybir.dt.uint8
DR = mybir.MatmulPerfMode.DoubleRow


def _emit(tc):
    nc = tc.nc
    qTx = nc.dram_tensor("qTx", [NSB, 128, KC, 512], F16, kind="ExternalInput").ap()
    kTx = nc.dram_tensor("kTx", [NSB, 128, KC, 512], F16, kind="ExternalInput").ap()
    vTx = nc.dram_tensor("vTx", [NSB, 128, KC, 512], F16, kind="ExternalInput").ap()
    wqT = nc.dram_tensor("wqT", [DM, HPC * DK], F16, kind="ExternalInput").ap()
    wkT = nc.dram_tensor("wkT", [DM, HPC * DK], F16, kind="ExternalInput").ap()
    wvT = nc.dram_tensor("wvT", [DM, HPC * DK], F16, kind="ExternalInput").ap()
    woT = nc.dram_tensor("woT", [HPC * DK, DM], F16, kind="ExternalInput").ap()
    bq = nc.dram_tensor("bq", [HPC * DK, 1], F32, kind="ExternalInput").ap()
    bk = nc.dram_tensor("bk", [HPC * DK, 1], F32, kind="ExternalInput").ap()
    bv = nc.dram_tensor("bv", [HPC * DK, 1], F32, kind="ExternalInput").ap()
    out_p = nc.dram_tensor("out_p", [S, DM], F16, kind="ExternalOutput").ap()
    den_d = nc.dram_tensor("den_d", [NSB * HPC, 512], F32, kind="Internal").ap()

    with (
        tc.tile_pool(name="const", bufs=1) as const,
        tc.tile_pool(name="heads", bufs=1) as heads,
        tc.tile_pool(name="xts", bufs=10) as xts,
        tc.tile_pool(name="work", bufs=3) as work,
        tc.tile_pool(name="norm", bufs=4) as norm,
    ):
        # ---- constants -------------------------------------------------
        w_q = const.tile([128, KC, HPC * DK], F16, tag="w_q")
        w_k = const.tile([128, KC, HPC * DK], F16, tag="w_k")
        w_v = const.tile([128, KC, HPC * DK], F16, tag="w_v")
        nc.sync.dma_start(w_k[:], wkT.rearrange("(c p) m -> p c m", p=128))
        wo01 = const.tile([128, DM], F16, tag="wo01")
        wo2 = const.tile([DK, DM], F16, tag="wo2")
        bq01 = const.tile([128, 1], F32, tag="bq01")
        bq2 = const.tile([DK, 1], F32, tag="bq2")
        bk01 = const.tile([128, 1], F32, tag="bk01")
        bk2 = const.tile([DK, 1], F32, tag="bk2")
        nc.sync.dma_start(bk01[:], bk[0:128, :])
        nc.sync.dma_start(bk2[:], bk[128:192, :])
        # v-bias broadcast to all 128 partitions: bvb[p, j] = bv[j]
        bvb = const.tile([128, HPC * DK], F32, tag="bvb")
        bv_bcast = bass.AP(
            tensor=bv.tensor, offset=bv.offset, ap=[[0, 128]] + list(bv.ap)
        )

        # preload the exp activation table during the projection phase
        warm = const.tile([1, 1], F32, tag="warm")
        nc.vector.memset(warm[:], 0.0)
        nc.scalar.activation(warm[:], warm[:], mybir.ActivationFunctionType.Exp)

        # ---- per-head persistent tensors ------------------------------
        # qT2/kT2: [128, S] fp16, rows 0:64 and 64:128 both hold head's
        # qT/kT (duplicated so row-tiled matmul pairs can stream from
        # either partition half).
        qT2 = [heads.tile([128, S], F16, tag=f"qT2_{h}", name=f"qT2_{h}") for h in range(HPC)]
        kT2 = [heads.tile([128, S], F16, tag=f"kT2_{h}", name=f"kT2_{h}") for h in range(HPC)]
        # v_dr: [128, NKV, 80] fp8; [p, g, 0:64] = v of kv chunk g (seq pos p
        # on partitions), col 64 = 1.0 (denominator column), 65:80 pad.
        v_dr = [heads.tile([128, NKV, 80], F8, tag=f"v_dr_{h}", name=f"v_dr_{h}") for h in range(HPC)]
        for h in range(HPC):
            nc.vector.memset(v_dr[h][:], 1.0)
        # normalized context, transposed: ctx01 rows 0:64 = head 0, rows
        # 64:128 = head 1; ctx2 = head 2. Together the lhsT of the output
        # projection.
        ctx01 = heads.tile([128, S], F16, tag="ctx01")
        ctx2 = heads.tile([64, S], F16, tag="ctx2")

        def dma_in(tile_ap, src, i):
            eng = nc.sync if i % 2 == 0 else nc.scalar
            eng.dma_start(tile_ap, src)

        # ---- projections: k --------------------------------------------
        with tc.tile_pool(name="pp", bufs=2, space=bass.MemorySpace.PSUM) as pp:
            kxs = []
            for sb in range(NSB):
                kx = xts.tile([128, KC, 512], F16, tag="xx", name=f"kx_{sb}")
                dma_in(kx[:], kTx[sb], sb)
                kxs.append(kx)
            for sb in range(NSB):
                sq = bass.ts(sb, 512)
                k01 = pp.tile([128, 512], F32, tag="k01")
                k2 = pp.tile([DK, 512], F32, tag="k2")
                for kc in range(KC):
                    st = dict(start=(kc == 0), stop=(kc == KC - 1))
                    xsl = kxs[sb][:, kc, :]
                    nc.tensor.matmul(k01[:], w_k[:, kc, 0:128], xsl, **st)
                    nc.tensor.matmul(k2[:], w_k[:, kc, 128:192], xsl, **st)
                nc.vector.tensor_scalar_add(kT2[0][0:64, sq], k01[0:64, :], bk01[0:64, :])
                nc.vector.tensor_scalar_add(kT2[1][0:64, sq], k01[64:128, :], bk01[64:128, :])
                nc.vector.tensor_scalar_add(kT2[2][0:64, sq], k2[:], bk2[:])
                for h in range(HPC):
                    nc.sync.dma_start(kT2[h][64:128, sq], kT2[h][0:64, sq])

        # v-proj inputs early so the DMA stream stays busy
        nc.sync.dma_start(w_v[:], wvT.rearrange("(c p) m -> p c m", p=128))
        nc.sync.dma_start(bvb[:], bv_bcast)

        # ---- projections: v -------------------------------------------
        # v rows (seq) on partitions: out tile [128 seq, 192] per kv chunk.
        with tc.tile_pool(name="vp", bufs=4, space=bass.MemorySpace.PSUM) as vp:
            vxs = []
            for sb in range(NSB):
                vx = xts.tile([128, KC, 512], F16, tag="xx", name=f"vx_{sb}")
                dma_in(vx[:], vTx[sb], sb)
                vxs.append(vx)
            for sb in range(NSB):
                for ss in range(4):  # kv chunk index = 4*sb + ss
                    vps = vp.tile([128, HPC * DK], F32, tag="vps")
                    for kc in range(KC):
                        nc.tensor.matmul(
                            vps[:],
                            vxs[sb][:, kc, bass.ds(ss * 128, 128)],
                            w_v[:, kc, :],
                            start=(kc == 0),
                            stop=(kc == KC - 1),
                        )
                    g = 4 * sb + ss
                    for h in range(HPC):
                        nc.vector.tensor_add(
                            v_dr[h][:, g, 0:64],
                            vps[:, bass.ts(h, 64)],
                            bvb[:, bass.ts(h, 64)],
                        )

        # remaining constant loads
        nc.sync.dma_start(w_q[:], wqT.rearrange("(c p) m -> p c m", p=128))
        nc.sync.dma_start(bq01[:], bq[0:128, :])
        nc.sync.dma_start(bq2[:], bq[128:192, :])
        nc.sync.dma_start(wo01[:], woT[0:128, :])
        nc.sync.dma_start(wo2[:], woT[128:192, :])

        # ---- attention + output projection ----------------------------
        # q chunks outer, heads inner. Scores land in [128, 1024] PSUM tiles
        # (one kv-chunk pair per exp, 3-deep ring shared with the q-proj
        # accumulator); P is written as fp8 into flat [128, 1024] staging
        # tiles, consumed by one DoubleRow P@V matmul per pair, issued two
        # slots late so its exp is long finished. The next q chunk's
        # projection rides inside head 2's loop; the previous q chunk's
        # output projection inside head 0's. Each head's normalize
        # (reciprocal + scale) is deferred into the NEXT head's loop so the
        # denominator's DRAM-broadcast round trip never blocks the DVE queue.
        with (
            tc.tile_pool(name="sp", bufs=3, space=bass.MemorySpace.PSUM) as sp,
            tc.tile_pool(name="bigp", bufs=2, space=bass.MemorySpace.PSUM) as bigp,
        ):
            def op_chain(qc, i):
                # one eighth of q-chunk qc's output projection
                qs, half = i // 2, i % 2
                n0, nw = (0, 512) if half == 0 else (512, 256)
                qsl = bass.ds(qc * 512 + qs * 128, 128)
                op = bigp.tile([128, 512], F32, tag="big", name=f"op_{qc}_{i}")
                nc.tensor.matmul(
                    op[:, 0:nw], ctx01[:, qsl], wo01[:, n0 : n0 + nw],
                    start=True, stop=False,
                )
                nc.tensor.matmul(
                    op[:, 0:nw], ctx2[:, qsl], wo2[:, n0 : n0 + nw],
                    start=False, stop=True,
                )
                ob = work.tile([128, 512], F16, tag="ob", name=f"ob_{qc}_{i}")
                nc.scalar.copy(ob[:, 0:nw], op[:, 0:nw])
                nc.sync.dma_start(out_p[qsl, n0 : n0 + nw], ob[:, 0:nw])

            qp_state = {}

            def qproj_step(qc, kc):
                if kc == 0:
                    qp_state[qc] = sp.tile([128, 1024], F32, tag="sT",
                                           name=f"qp_{qc}")
                    qx = xts.tile([128, KC, 512], F16, tag="xx",
                                  name=f"qx_{qc}")
                    nc.sync.dma_start(qx[:], qTx[qc])
                    qp_state[f"x{qc}"] = qx
                qp = qp_state[qc]
                qx = qp_state[f"x{qc}"]
                st = dict(start=(kc == 0), stop=(kc == KC - 1))
                xsl = qx[:, kc, :]
                nc.tensor.matmul(qp[:, 0:512], w_q[:, kc, 0:128], xsl, **st)
                nc.tensor.matmul(qp[0:64, 512:1024], w_q[:, kc, 128:192], xsl, **st)

            def qproj_drain(qc):
                sq = bass.ts(qc, 512)
                qp = qp_state.pop(qc)
                qp_state.pop(f"x{qc}")
                nc.vector.tensor_scalar_add(qT2[0][0:64, sq], qp[0:64, 0:512], bq01[0:64, :])
                nc.vector.tensor_scalar_add(qT2[1][0:64, sq], qp[64:128, 0:512], bq01[64:128, :])
                nc.vector.tensor_scalar_add(qT2[2][0:64, sq], qp[0:64, 512:1024], bq2[:])
                for h in range(HPC):
                    nc.sync.dma_start(qT2[h][64:128, sq], qT2[h][0:64, sq])

            pending_norm = [None]

            def flush_norm():
                if pending_norm[0] is not None:
                    pending_norm[0]()
                    pending_norm[0] = None

            for kc in range(KC):
                qproj_step(0, kc)
            qproj_drain(0)

            prev = [None]  # (ctx_mm, finish) of the previous head

            def finish_head(qc, h, ctx):
                # denominator row -> SBUF (on ACT) -> DRAM -> stride-0
                # broadcast back to 64 partitions; reciprocal + scale are
                # deferred further (flush_norm) so the round trip never
                # blocks the DVE queue.
                sq = bass.ts(qc, 512)
                den_row = norm.tile([1, 512], F32, tag="den_row")
                nc.scalar.copy(den_row[:], ctx[64:65, :])
                di = qc * HPC + h
                nc.sync.dma_start(den_d[di : di + 1, :], den_row[:])
                den = norm.tile([64, 512], F32, tag="den")
                dsrc = den_d[di : di + 1, :]
                den_bcast = bass.AP(
                    tensor=dsrc.tensor,
                    offset=dsrc.offset,
                    ap=[[0, 64]] + list(dsrc.ap[1:]),
                )
                nc.sync.dma_start(den[:], den_bcast)

                def normalize():
                    rec = norm.tile([64, 512], F32, tag="rec")
                    nc.vector.reciprocal_approx_fast(out=rec[:], in_=den[:])
                    if h == 0:
                        nc.vector.tensor_mul(ctx01[0:64, sq], ctx[0:64, :], rec[:])
                    elif h == 1:
                        nc.vector.tensor_mul(ctx01[64:128, sq], ctx[0:64, :], rec[:])
                    else:
                        nc.vector.tensor_mul(ctx2[:, sq], ctx[0:64, :], rec[:])

                pending_norm[0] = normalize

            for qc in range(NSB):
                sq = bass.ts(qc, 512)
                for h in range(HPC):
                    ctx_t = bigp.tile([128, 512], F32, tag="big")
                    ctx = ctx_t[0:65, :]
                    pts = {}

                    def ctx_mm(g, ctx=ctx, h=h, pts=pts):
                        nc.tensor.matmul(
                            ctx,
                            v_dr[h][:, 2 * g : 2 * g + 2, 0:65],
                            pts.pop(g)[:].rearrange("p (a b) -> p a b", a=2),
                            start=(g == 0), stop=(g == 15),
                            perf_mode=DR,
                        )

                    for g in range(16):  # kv-chunk pairs
                        # previous head's last two P@V matmuls ride in this
                        # head's first two slots (no PE drain at the boundary)
                        if g <= 2 and prev[0] is not None:
                            prev[0][0](13 + g)
                            if g == 2:
                                prev[0][1]()
                                prev[0] = None
                        if g >= 3:
                            ctx_mm(g - 3)
                        sT = sp.tile([128, 1024], F32, tag="sT")
                        for j in (0, 1):
                            c = 2 * g + j
                            lo = 64 * j
                            nc.tensor.matmul(
                                sT[:, bass.ts(j, 512)],
                                kT2[h][lo : lo + 64, bass.ts(c, 128)],
                                qT2[h][lo : lo + 64, sq],
                            )
                        pt = work.tile([128, 1024], F8, tag="pt", bufs=8,
                                       name=f"pt_{qc}_{h}_{g}")
                        pts[g] = pt
                        if g in DVE_PAIRS:
                            nc.vector.tensor_scalar(
                                pt.bitcast(U8)[:], sT[:], A_SCH, B_SCH,
                                mybir.AluOpType.mult, mybir.AluOpType.add,
                            )
                        else:
                            nc.scalar.activation(
                                pt[:], sT[:], mybir.ActivationFunctionType.Exp,
                                scale=SCALE,
                            )
                        if g == 6:
                            flush_norm()
                        # previous q-chunk's output projection, spread across
                        # heads 0 and 1
                        if h <= 1 and qc > 0 and g in (7, 9, 11, 13):
                            op_chain(qc - 1, 4 * h + (g - 7) // 2)
                        # next q-chunk's projection inside head 2's loop
                        if h == 2 and qc + 1 < NSB:
                            if 1 <= g <= 6:
                                qproj_step(qc + 1, g - 1)
                            elif g == 7:
                                qproj_drain(qc + 1)
                    prev[0] = (ctx_mm, lambda qc=qc, h=h, ctx=ctx: finish_head(qc, h, ctx))
            # drain the last head
            ctx_mm_f, fin = prev[0]
            ctx_mm_f(13)
            ctx_mm_f(14)
            ctx_mm_f(15)
            fin()
            flush_norm()
            # last q-chunk's output projection
            for i in range(8):
                op_chain(NSB - 1, i)


_NC_CACHE = {}


def _build():
    if "nc" not in _NC_CACHE:
        nc = bacc.Bacc(
            "TRN2", target_bir_lowering=False, debug=False, num_devices=NC_CORES
        )
        with tile.TileContext(nc) as tc:
            _emit(tc)
        nc.compile()
        _NC_CACHE["nc"] = nc
    return _NC_CACHE["nc"]


def _tile_xT(x):
    # x: [S, DM] fp32 -> x.T tiled as [NSB, 128, KC*512] fp16: one contiguous
    # 768 KiB block per 512-seq chunk (3 KiB per partition line).
    xT = np.ascontiguousarray(x.T).astype(np.float16)  # [DM, S]
    t = xT.reshape(KC, 128, NSB, 512).transpose(2, 1, 0, 3)
    return np.ascontiguousarray(t)


def make_in_maps(query, key, value, wq, bq, wk, bk, wv, bv, wo, bo):
    query = np.asarray(query)
    key = np.asarray(key)
    value = np.asarray(value)
    wq, bq, wk, bk, wv, bv, wo, bo = (
        np.asarray(a) for a in (wq, bq, wk, bk, wv, bv, wo, bo)
    )
    in_maps = []
    for c in range(NC_CORES):
        b = c // 4
        hs = (c % 4) * HPC * DK
        he = hs + HPC * DK
        in_maps.append(
            {
                "qTx": _tile_xT(query[b]),
                "kTx": _tile_xT(key[b]),
                "vTx": _tile_xT(value[b]),
                "wqT": np.ascontiguousarray(wq[hs:he, :].T).astype(np.float16),
                "wkT": np.ascontiguousarray(wk[hs:he, :].T).astype(np.float16),
                "wvT": np.ascontiguousarray(wv[hs:he, :].T).astype(np.float16),
                "woT": np.ascontiguousarray(wo[:, hs:he].T).astype(np.float16),
                "bq": bq[hs:he].reshape(-1, 1).astype(np.float32),
                "bk": bk[hs:he].reshape(-1, 1).astype(np.float32),
                "bv": bv[hs:he].reshape(-1, 1).astype(np.float32),
            }
        )
    return in_maps


def combine_outputs(results, bo):
    parts = [results[c]["out_p"].astype(np.float32) for c in range(NC_CORES)]
    out0 = parts[0] + parts[1] + parts[2] + parts[3]
    out1 = parts[4] + parts[5] + parts[6] + parts[7]
    out = np.stack([out0, out1]) + np.asarray(bo)[None, None, :]
    return out.astype(np.float32)


def run_on_hw(in_maps, **kw):
    nc = _build()
    return run_bass_kernel_spmd(nc, in_maps, list(range(NC_CORES)), **kw)


def kernel(query, key, value, wq, bq, wk, bk, wv, bv, wo, bo):
    in_maps = make_in_maps(query, key, value, wq, bq, wk, bk, wv, bv, wo, bo)
    res = run_on_hw(in_maps)
    return combine_outputs(res.results, bo)


# revision 14
# speedup vs baseline: 1.0777x; 1.0036x over previous
"""Multi-head attention (B=2, S=4096, D=768, H=12) on 8 TRN2 NeuronCores.

Sharding: 24 (batch, head) pairs -> 3 heads per core. Cores 0-3 take batch 0,
cores 4-7 take batch 1. Each core computes q/k/v projections for its 3 heads,
flash-style attention (scores kept transposed [kv, q] so exp can run straight
out of PSUM), and a partial output projection over its 192 contraction rows.
The host sums the 4 partial outputs per batch and adds the output bias.

v2 structure:
- Inputs land as one 768 KiB DMA per 512-seq block (3 KiB/partition lines),
  alternating between the two HWDGE issue engines (sync / scalar).
- All projections (k, v, q) run up front; attention then owns all 8 PSUM
  banks: 2x [128,1536] score tiles + ctx accumulator + out-proj staging.
- Softmax exp is split across two engines: scalar ACT computes true exp
  (fp8 out), and the vector engine computes a Schraudolph-style exp for
  ~1/3 of the kv chunks: u8 = round(s*A + B) bit-cast as fp8e4m3, a
  piecewise-linear 2^x approximation (~3% max err, zero-mean).
- P@V runs in fp8 with DoubleRow perf mode: each matmul contracts TWO kv
  chunks (virtual 256-deep array), halving P@V tensor-engine time. The
  denominator falls out via a ones-column appended to V.
"""

import sys

sys.path.insert(0, "/opt/trn_rl_repo")

import numpy as np  # noqa: E402

from concourse import bacc, bass, mybir, tile  # noqa: E402
from concourse.bass_utils import run_bass_kernel_spmd  # noqa: E402

S = 4096
DM = 768
DK = 64
HPC = 3  # heads per core
NC_CORES = 8
KC = DM // 128  # 6 contraction chunks for projections
NSB = S // 512  # 8 seq blocks (projection N / attention q chunks)
NKV = S // 128  # 32 kv chunks
SCALE = 1.0 / np.sqrt(DK)
# Schraudolph fp8e4m3 exp: u8 = round(s * A + B), bits viewed as fp8.
A_SCH = float(8.0 * np.log2(np.e) * SCALE)
B_SCH = float(56.0 - 0.3443)
DVE_PAIRS = (1, 3, 5, 7, 9, 11, 13, 15)  # exp pairs computed on the vector engine

F16 = mybir.dt.float16
F32 = mybir.dt.float32
F8 = mybir.dt.float8e4
U8 = mybir.dt.uint8
DR = mybir.MatmulPerfMode.DoubleRow


def _emit(tc):
    nc = tc.nc
    qTx = nc.dram_tensor("qTx", [NSB, 128, KC, 512], F16, kind="ExternalInput").ap()
    kTx = nc.dram_tensor("kTx", [NSB, 128, KC, 512], F16, kind="ExternalInput").ap()
    vTx = nc.dram_tensor("vTx", [NSB, 128, KC, 512], F16, kind="ExternalInput").ap()
    wqT = nc.dram_tensor("wqT", [DM, HPC * DK], F16, kind="ExternalInput").ap()
    wkT = nc.dram_tensor("wkT", [DM, HPC * DK], F16, kind="ExternalInput").ap()
    wvT = nc.dram_tensor("wvT", [DM, HPC * DK], F16, kind="ExternalInput").ap()
    woT = nc.dram_tensor("woT", [HPC * DK, DM], F16, kind="ExternalInput").ap()
    bq = nc.dram_tensor("bq", [HPC * DK, 1], F32, kind="ExternalInput").ap()
    bk = nc.dram_tensor("bk", [HPC * DK, 1], F32, kind="ExternalInput").ap()
    bv = nc.dram_tensor("bv", [HPC * DK, 1], F32, kind="ExternalInput").ap()
    out_p = nc.dram_tensor("out_p", [S, DM], F16, kind="ExternalOutput").ap()
    den_d = nc.dram_tensor("den_d", [NSB * HPC, 512], F32, kind="Internal").ap()

    with (
        tc.tile_pool(name="const", bufs=1) as const,
        tc.tile_pool(name="heads", bufs=1) as heads,
        tc.tile_pool(name="xts", bufs=10) as xts,
        tc.tile_pool(name="work", bufs=3) as work,
        tc.tile_pool(name="norm", bufs=4) as norm,
    ):
        # ---- constants -------------------------------------------------
        w_q = const.tile([128, KC, HPC * DK], F16, tag="w_q")
        w_k = const.tile([128, KC, HPC * DK], F16, tag="w_k")
        w_v = const.tile([128, KC, HPC * DK], F16, tag="w_v")
        nc.sync.dma_start(w_k[:], wkT.rearrange("(c p) m -> p c m", p=128))
        wo01 = const.tile([128, DM], F16, tag="wo01")
        wo2 = const.tile([DK, DM], F16, tag="wo2")
        bq01 = const.tile([128, 1], F32, tag="bq01")
        bq2 = const.tile([DK, 1], F32, tag="bq2")
        bk01 = const.tile([128, 1], F32, tag="bk01")
        bk2 = const.tile([DK, 1], F32, tag="bk2")
        nc.sync.dma_start(bk01[:], bk[0:128, :])
        nc.sync.dma_start(bk2[:], bk[128:192, :])
        # v-bias broadcast to all 128 partitions: bvb[p, j] = bv[j]
        bvb = const.tile([128, HPC * DK], F32, tag="bvb")
        bv_bcast = bass.AP(
            tensor=bv.tensor, offset=bv.offset, ap=[[0, 128]] + list(bv.ap)
        )

        # preload the exp activation table during the projection phase
        warm = const.tile([1, 1], F32, tag="warm")
        nc.vector.memset(warm[:], 0.0)
        nc.scalar.activation(warm[:], warm[:], mybir.ActivationFunctionType.Exp)

        # ---- per-head persistent tensors ------------------------------
        # qT2/kT2: [128, S] fp16, rows 0:64 and 64:128 both hold head's
        # qT/kT (duplicated so row-tiled matmul pairs can stream from
        # either partition half).
        qT2 = [heads.tile([128, S], F16, tag=f"qT2_{h}", name=f"qT2_{h}") for h in range(HPC)]
        kT2 = [heads.tile([128, S], F16, tag=f"kT2_{h}", name=f"kT2_{h}") for h in range(HPC)]
        # v_dr: [128, NKV, 80] fp8; [p, g, 0:64] = v of kv chunk g (seq pos p
        # on partitions), col 64 = 1.0 (denominator column), 65:80 pad.
        v_dr = [heads.tile([128, NKV, 80], F8, tag=f"v_dr_{h}", name=f"v_dr_{h}") for h in range(HPC)]
        for h in range(HPC):
            nc.vector.memset(v_dr[h][:], 1.0)
        # normalized context, transposed: ctx01 rows 0:64 = head 0, rows
        # 64:128 = head 1; ctx2 = head 2. Together the lhsT of the output
        # projection.
        ctx01 = heads.tile([128, S], F16, tag="ctx01")
        ctx2 = heads.tile([64, S], F16, tag="ctx2")

        def dma_in(tile_ap, src, i):
            eng = nc.sync if i % 2 == 0 else nc.scalar
            eng.dma_start(tile_ap, src)

        # ---- projections: k --------------------------------------------
        with tc.tile_pool(name="pp", bufs=2, space=bass.MemorySpace.PSUM) as pp:
            kxs = []
            for sb in range(NSB):
                kx = xts.tile([128, KC, 512], F16, tag="xx", name=f"kx_{sb}")
                dma_in(kx[:], kTx[sb], sb)
                kxs.append(kx)
            for sb in range(NSB):
                sq = bass.ts(sb, 512)
                k01 = pp.tile([128, 512], F32, tag="k01")
                k2 = pp.tile([DK, 512], F32, tag="k2")
                for kc in range(KC):
                    st = dict(start=(kc == 0), stop=(kc == KC - 1))
                    xsl = kxs[sb][:, kc, :]
                    nc.tensor.matmul(k01[:], w_k[:, kc, 0:128], xsl, **st)
                    nc.tensor.matmul(k2[:], w_k[:, kc, 128:192], xsl, **st)
                nc.vector.tensor_scalar_add(kT2[0][0:64, sq], k01[0:64, :], bk01[0:64, :])
                nc.vector.tensor_scalar_add(kT2[1][0:64, sq], k01[64:128, :], bk01[64:128, :])
                nc.vector.tensor_scalar_add(kT2[2][0:64, sq], k2[:], bk2[:])
                for h in range(HPC):
                    nc.sync.dma_start(kT2[h][64:128, sq], kT2[h][0:64, sq])

        # v-proj inputs early so the DMA stream stays busy
        nc.sync.dma_start(w_v[:], wvT.rearrange("(c p) m -> p c m", p=128))
        nc.sync.dma_start(bvb[:], bv_bcast)

        # ---- projections: v -------------------------------------------
        # v rows (seq) on partitions: out tile [128 seq, 192] per kv chunk.
        with tc.tile_pool(name="vp", bufs=4, space=bass.MemorySpace.PSUM) as vp:
            vxs = []
            for sb in range(NSB):
                vx = xts.tile([128, KC, 512], F16, tag="xx", name=f"vx_{sb}")
                dma_in(vx[:], vTx[sb], sb)
                vxs.append(vx)
            for sb in range(NSB):
                for ss in range(4):  # kv chunk index = 4*sb + ss
                    vps = vp.tile([128, HPC * DK], F32, tag="vps")
                    for kc in range(KC):
                        nc.tensor.matmul(
                            vps[:],
                            vxs[sb][:, kc, bass.ds(ss * 128, 128)],
                            w_v[:, kc, :],
                            start=(kc == 0),
                            stop=(kc == KC - 1),
                        )
                    g = 4 * sb + ss
                    for h in range(HPC):
                        nc.vector.tensor_add(
                            v_dr[h][:, g, 0:64],
                            vps[:, bass.ts(h, 64)],
                            bvb[:, bass.ts(h, 64)],
                        )

        # remaining constant loads
        nc.sync.dma_start(w_q[:], wqT.rearrange("(c p) m -> p c m", p=128))
        nc.sync.dma_start(bq01[:], bq[0:128, :])
        nc.sync.dma_start(bq2[:], bq[128:192, :])
        nc.sync.dma_start(wo01[:], woT[0:128, :])
        nc.sync.dma_start(wo2[:], woT[128:192, :])

        # ---- projections: q -------------------------------------------
        with tc.tile_pool(name="qp", bufs=2, space=bass.MemorySpace.PSUM) as qpool:
            qxs = []
            for qc in range(NSB):
                qx = xts.tile([128, KC, 512], F16, tag="xx", name=f"qx_{qc}")
                dma_in(qx[:], qTx[qc], qc)
                qxs.append(qx)
            for qc in range(NSB):
                sq = bass.ts(qc, 512)
                qp = qpool.tile([128, 1024], F32, tag="qp")
                for kc in range(KC):
                    st = dict(start=(kc == 0), stop=(kc == KC - 1))
                    xsl = qxs[qc][:, kc, :]
                    nc.tensor.matmul(qp[:, 0:512], w_q[:, kc, 0:128], xsl, **st)
                    nc.tensor.matmul(qp[0:64, 512:1024], w_q[:, kc, 128:192], xsl, **st)
                nc.vector.tensor_scalar_add(qT2[0][0:64, sq], qp[0:64, 0:512], bq01[0:64, :])
                nc.vector.tensor_scalar_add(qT2[1][0:64, sq], qp[64:128, 0:512], bq01[64:128, :])
                nc.vector.tensor_scalar_add(qT2[2][0:64, sq], qp[0:64, 512:1024], bq2[:])
                for h in range(HPC):
                    nc.sync.dma_start(qT2[h][64:128, sq], qT2[h][0:64, sq])

        # ---- attention + output projection ----------------------------
        # q chunks outer, heads inner. Scores land in [128, 1024] PSUM tiles
        # (one kv-chunk pair per exp, 3-deep ring shared with the q-proj
        # accumulator); P is written as fp8 into flat [128, 1024] staging
        # tiles, consumed by one DoubleRow P@V matmul per pair, issued two
        # slots late so its exp is long finished. The next q chunk's
        # projection rides inside head 2's loop; the previous q chunk's
        # output projection inside head 0's. Each head's normalize
        # (reciprocal + scale) is deferred into the NEXT head's loop so the
        # denominator's DRAM-broadcast round trip never blocks the DVE queue.
        with (
            tc.tile_pool(name="sp", bufs=3, space=bass.MemorySpace.PSUM) as sp,
            tc.tile_pool(name="bigp", bufs=2, space=bass.MemorySpace.PSUM) as bigp,
        ):
            def op_chain(qc, i):
                # one eighth of q-chunk qc's output projection
                qs, half = i // 2, i % 2
                n0, nw = (0, 512) if half == 0 else (512, 256)
                qsl = bass.ds(qc * 512 + qs * 128, 128)
                op = bigp.tile([128, 512], F32, tag="big", name=f"op_{qc}_{i}")
                nc.tensor.matmul(
                    op[:, 0:nw], ctx01[:, qsl], wo01[:, n0 : n0 + nw],
                    start=True, stop=False,
                )
                nc.tensor.matmul(
                    op[:, 0:nw], ctx2[:, qsl], wo2[:, n0 : n0 + nw],
                    start=False, stop=True,
                )
                ob = work.tile([128, 512], F16, tag="ob", name=f"ob_{qc}_{i}")
                nc.scalar.copy(ob[:, 0:nw], op[:, 0:nw])
                nc.sync.dma_start(out_p[qsl, n0 : n0 + nw], ob[:, 0:nw])

            pending_norm = [None]

            def flush_norm():
                if pending_norm[0] is not None:
                    pending_norm[0]()
                    pending_norm[0] = None

            prev = [None]  # (ctx_mm, finish) of the previous head

            def finish_head(qc, h, ctx):
                # denominator row -> SBUF (on ACT) -> DRAM -> stride-0
                # broadcast back to 64 partitions; reciprocal + scale are
                # deferred further (flush_norm) so the round trip never
                # blocks the DVE queue.
                sq = bass.ts(qc, 512)
                den_row = norm.tile([1, 512], F32, tag="den_row")
                nc.scalar.copy(den_row[:], ctx[64:65, :])
                di = qc * HPC + h
                nc.sync.dma_start(den_d[di : di + 1, :], den_row[:])
                den = norm.tile([64, 512], F32, tag="den")
                dsrc = den_d[di : di + 1, :]
                den_bcast = bass.AP(
                    tensor=dsrc.tensor,
                    offset=dsrc.offset,
                    ap=[[0, 64]] + list(dsrc.ap[1:]),
                )
                nc.sync.dma_start(den[:], den_bcast)

                def normalize():
                    rec = norm.tile([64, 512], F32, tag="rec")
                    nc.vector.reciprocal_approx_fast(out=rec[:], in_=den[:])
                    if h == 0:
                        nc.vector.tensor_mul(ctx01[0:64, sq], ctx[0:64, :], rec[:])
                    elif h == 1:
                        nc.vector.tensor_mul(ctx01[64:128, sq], ctx[0:64, :], rec[:])
                    else:
                        nc.vector.tensor_mul(ctx2[:, sq], ctx[0:64, :], rec[:])

                pending_norm[0] = normalize

            for qc in range(NSB):
                sq = bass.ts(qc, 512)
                for h in range(HPC):
                    ctx_t = bigp.tile([128, 512], F32, tag="big")
                    ctx = ctx_t[0:65, :]
                    pts = {}

                    def ctx_mm(g, ctx=ctx, h=h, pts=pts):
                        nc.tensor.matmul(
                            ctx,
                            v_dr[h][:, 2 * g : 2 * g + 2, 0:65],
                            pts.pop(g)[:].rearrange("p (a b) -> p a b", a=2),
                            start=(g == 0), stop=(g == 15),
                            perf_mode=DR,
                        )

                    for g in range(16):  # kv-chunk pairs
                        # previous head's last two P@V matmuls ride in this
                        # head's first two slots (no PE drain at the boundary)
                        if g <= 2 and prev[0] is not None:
                            prev[0][0](13 + g)
                            if g == 2:
                                prev[0][1]()
                                prev[0] = None
                        if g >= 3:
                            ctx_mm(g - 3)
                        sT = sp.tile([128, 1024], F32, tag="sT")
                        for j in (0, 1):
                            c = 2 * g + j
                            lo = 64 * j
                            nc.tensor.matmul(
                                sT[:, bass.ts(j, 512)],
                                kT2[h][lo : lo + 64, bass.ts(c, 128)],
                                qT2[h][lo : lo + 64, sq],
                            )
                        pt = work.tile([128, 1024], F8, tag="pt", bufs=8,
                                       name=f"pt_{qc}_{h}_{g}")
                        pts[g] = pt
                        if g in DVE_PAIRS:
                            nc.vector.tensor_scalar(
                                pt.bitcast(U8)[:], sT[:], A_SCH, B_SCH,
                                mybir.AluOpType.mult, mybir.AluOpType.add,
                            )
                        else:
                            nc.scalar.activation(
                                pt[:], sT[:], mybir.ActivationFunctionType.Exp,
                                scale=SCALE,
                            )
                        if g == 6:
                            flush_norm()
                        # previous q-chunk's output projection, spread across
                        # heads 0 and 1
                        if h <= 1 and qc > 0 and g in (7, 9, 11, 13):
                            op_chain(qc - 1, 4 * h + (g - 7) // 2)
                    prev[0] = (ctx_mm, lambda qc=qc, h=h, ctx=ctx: finish_head(qc, h, ctx))
            # drain the last head
            ctx_mm_f, fin = prev[0]
            ctx_mm_f(13)
            ctx_mm_f(14)
            ctx_mm_f(15)
            fin()
            flush_norm()
            # last q-chunk's output projection
            for i in range(8):
                op_chain(NSB - 1, i)


_NC_CACHE = {}


def _build():
    if "nc" not in _NC_CACHE:
        nc = bacc.Bacc(
            "TRN2", target_bir_lowering=False, debug=False, num_devices=NC_CORES
        )
        with tile.TileContext(nc) as tc:
            _emit(tc)
        nc.compile()
        _NC_CACHE["nc"] = nc
    return _NC_CACHE["nc"]


def _tile_xT(x):
    # x: [S, DM] fp32 -> x.T tiled as [NSB, 128, KC*512] fp16: one contiguous
    # 768 KiB block per 512-seq chunk (3 KiB per partition line).
    xT = np.ascontiguousarray(x.T).astype(np.float16)  # [DM, S]
    t = xT.reshape(KC, 128, NSB, 512).transpose(2, 1, 0, 3)
    return np.ascontiguousarray(t)


def make_in_maps(query, key, value, wq, bq, wk, bk, wv, bv, wo, bo):
    query = np.asarray(query)
    key = np.asarray(key)
    value = np.asarray(value)
    wq, bq, wk, bk, wv, bv, wo, bo = (
        np.asarray(a) for a in (wq, bq, wk, bk, wv, bv, wo, bo)
    )
    in_maps = []
    for c in range(NC_CORES):
        b = c // 4
        hs = (c % 4) * HPC * DK
        he = hs + HPC * DK
        in_maps.append(
            {
                "qTx": _tile_xT(query[b]),
                "kTx": _tile_xT(key[b]),
                "vTx": _tile_xT(value[b]),
                "wqT": np.ascontiguousarray(wq[hs:he, :].T).astype(np.float16),
                "wkT": np.ascontiguousarray(wk[hs:he, :].T).astype(np.float16),
                "wvT": np.ascontiguousarray(wv[hs:he, :].T).astype(np.float16),
                "woT": np.ascontiguousarray(wo[:, hs:he].T).astype(np.float16),
                "bq": bq[hs:he].reshape(-1, 1).astype(np.float32),
                "bk": bk[hs:he].reshape(-1, 1).astype(np.float32),
                "bv": bv[hs:he].reshape(-1, 1).astype(np.float32),
            }
        )
    return in_maps


def combine_outputs(results, bo):
    parts = [results[c]["out_p"].astype(np.float32) for c in range(NC_CORES)]
    out0 = parts[0] + parts[1] + parts[2] + parts[3]
    out1 = parts[4] + parts[5] + parts[6] + parts[7]
    out = np.stack([out0, out1]) + np.asarray(bo)[None, None, :]
    return out.astype(np.float32)


def run_on_hw(in_maps, **kw):
    nc = _build()
    return run_bass_kernel_spmd(nc, in_maps, list(range(NC_CORES)), **kw)


def kernel(query, key, value, wq, bq, wk, bk, wv, bv, wo, bo):
    in_maps = make_in_maps(query, key, value, wq, bq, wk, bk, wv, bv, wo, bo)
    res = run_on_hw(in_maps)
    return combine_outputs(res.results, bo)
